# revision 17
# baseline (speedup 1.0000x reference)
"""Distributional (Gaussian-KL) attention on 8 TRN2 NeuronCores.

Math: for each head, the KL-based score decomposes as
    kl[q,k] = sum_d (Qm-Km)^2/(2Kv) + 0.5*(Qv/Kv - log(Qv/Kv) - 1)
            = Fq[q] . Fk[k] + r[k] + c[q]
with  Fq = [Qm^2+Qv ; -Qm],  Fk = [1/(2Kv) ; Km/Kv],
      r  = 0.5*sum_d (Km^2/Kv + log Kv),
and c[q] only shifts softmax logits per-row (drops out of softmax over k).
scores = -kl/sqrt(Dh); kl >= 0 and measured row-max kl/8 <= ~17, so
exp(scores) needs no max-shift.

Sharding: head-parallel. Core c owns heads {2c, 2c+1} == feature columns
[128c, 128c+128) of every Q/K/V projection. The output projection needs
all heads, so the per-core attention outputs (feature-major, bf16, mu and
var packed side by side in one [128, 512] buffer) are AllGathered in a
SINGLE collective, after which core c computes output columns
[128c, 128c+128). Host concatenates + transposes.

Host-side prep (inside kernel(), numpy only): weights and inputs are
pre-cast to bf16 and pre-transposed/tiled into PE-friendly layouts, so
the device does no f32->bf16 casting and no input transposes at all.

Precision: all matmuls bf16 except the r_k bias path (true fp32 - r is
O(100+) and bf16 rounding there shifts logits by ~0.15; r is added to
the logits as a bf16 hi+lo pair via two rank-1 matmuls).

ACT table discipline (each table swap costs ~1.28us): the op order is
sigmoid x3 -> one wide Ln (+ t_lg Ln) -> exp (1/Kv, attention, final
softplus numerator) -> one final Ln.  softplus(x)=ln(sigmoid(-x)) up
front (sigmoid loaded first), but the OUTPUT softplus is computed as
ln(1+exp(x)) so it reuses the exp table that attention left resident.
1/Kv = exp(-ln Kv) on ACT replaces a slow DVE reciprocal.
"""

import numpy as np

import concourse.bass as bass
import concourse.mybir as mybir
import concourse.tile as tile
from concourse import bacc
from concourse.masks import make_identity
from concourse.bass_utils import run_bass_kernel_spmd

F32 = mybir.dt.float32
BF16 = mybir.dt.bfloat16
AF = mybir.ActivationFunctionType
ALU = mybir.AluOpType
AX = mybir.AxisListType

H, B, L, D = 16, 1, 256, 1024
Dh = D // H          # 64
NCORES = 8
CB = D // NCORES     # 128 feature columns per core (2 heads)
P = 128
LT = L // P          # 2 row tiles of the sequence
KT = D // P          # 8 contraction tiles
NW = 8               # weights in wcat: qv kv vv qm km vm | ov om

TRACE = False
TRACE_KWARGS = {}
LAST_RESULT = None

_prog_cache = {}


def ts(i, size):
    return slice(i * size, (i + 1) * size)


def build_program():
    nc = bacc.Bacc("TRN2", target_bir_lowering=False, debug=False,
                   num_devices=NCORES)

    # xcat[p, s, kt, l] = x_s[l, kt*128+p]; s: 0=var, 1=mu
    xcat_d = nc.dram_tensor("xcat", [P, 2, KT, L], BF16, kind="ExternalInput")
    # wcat[p, w, kt, m] = w[kt*128+p, m]
    wcat_d = nc.dram_tensor("wcat", [P, NW, KT, CB], BF16,
                            kind="ExternalInput")
    b_d = nc.dram_tensor("biases", [CB, 8], F32, kind="ExternalInput")
    out_mu_d = nc.dram_tensor("out_mu", [CB, L], F32, kind="ExternalOutput")
    out_var_d = nc.dram_tensor("out_var", [CB, L], F32, kind="ExternalOutput")

    with tile.TileContext(nc) as tc:
        _build(nc, tc, xcat_d, wcat_d, b_d, out_mu_d, out_var_d)
    nc.compile()
    return nc


def _build(nc, tc, xcat_d, wcat_d, b_d, out_mu_d, out_var_d):
    from contextlib import ExitStack
    ctx = ExitStack()
    with ctx:
        const = ctx.enter_context(tc.tile_pool(name="const", bufs=1))
        persist = ctx.enter_context(tc.tile_pool(name="persist", bufs=1))
        stage = ctx.enter_context(tc.tile_pool(name="stage", bufs=3))
        feat = ctx.enter_context(tc.tile_pool(name="feat", bufs=1))
        attnp = ctx.enter_context(tc.tile_pool(name="attnp", bufs=2))
        ps_proj = ctx.enter_context(
            tc.tile_pool(name="ps_proj", bufs=2, space="PSUM"))
        ps_tr = ctx.enter_context(
            tc.tile_pool(name="ps_tr", bufs=2, space="PSUM"))
        ps_s = ctx.enter_context(
            tc.tile_pool(name="ps_s", bufs=2, space="PSUM"))
        ps_small = ctx.enter_context(
            tc.tile_pool(name="ps_small", bufs=1, space="PSUM"))
        dram = ctx.enter_context(tc.tile_pool(name="dram", bufs=1,
                                              space="DRAM"))

        # ---------------- core-alignment barrier ------------------------
        # Cores launch ~11us apart (staggered). A tiny AllGather whose
        # output gates the bias tile aligns all instruction streams while
        # the input DMAs run, so the real gathers later see ~zero skew.
        zin = stage.tile([16, 4], F32, tag="zin", name="zin", bufs=1)
        nc.vector.memset(zin, 0.0)
        d_in = dram.tile([16, 4], F32, tag="d_in", name="d_in")
        nc.sync.dma_start(d_in[:, :], zin)
        d_out = dram.tile([P, 4], F32, tag="d_out", name="d_out",
                          addr_space="Shared")
        nc.gpsimd.collective_compute(
            "AllGather", ALU.bypass,
            replica_groups=[list(range(NCORES))],
            ins=[d_in[:].opt()],
            outs=[d_out[:].opt()],
        )
        # NB: issued on the gpsimd queue — on sync it would serialize the
        # x/w input DMAs behind the barrier collective's completion.
        z128 = const.tile([P, 4], F32, tag="z128", name="z128")
        nc.gpsimd.dma_start(z128, d_out[:, :])

        # ---------------- inputs: clean bf16 DMAs, finest useful order --
        x_sb = persist.tile([P, 2, KT, L], BF16, tag="x_sb", name="x_sb")
        nc.sync.dma_start(x_sb[:, 0], xcat_d.ap()[:, 0])     # var first
        w_sb = persist.tile([P, NW, KT, CB], BF16, tag="w_sb", name="w_sb")
        for wi in range(3):                                  # qkv_var
            nc.sync.dma_start(w_sb[:, wi:wi + 1], wcat_d.ap()[:, wi:wi + 1])
        nc.sync.dma_start(x_sb[:, 1], xcat_d.ap()[:, 1])     # mu
        for wi in range(3, 6):                               # qkv_mu
            nc.sync.dma_start(w_sb[:, wi:wi + 1], wcat_d.ap()[:, wi:wi + 1])
        nc.sync.dma_start(w_sb[:, 6:8], wcat_d.ap()[:, 6:8])  # wo_var, wo_mu

        # ---------------- constants -----------------------------------
        ident_b = const.tile([P, P], BF16, tag="ident_b", name="ident_b")
        make_identity(nc, ident_b)
        for _w in range(8):
            wmp = ps_tr.tile([P, P], F32, tag="tr", name="warm")
            nc.tensor.matmul(wmp, ident_b, ident_b, start=True, stop=True)
        half2 = const.tile([2, P], BF16, tag="half2", name="half2")
        nc.vector.memset(half2, 0.5)
        # ind_h[p, :] = 0.5 if p in head h else 0 (both cols; the fp32 r
        # matmul then yields r_h duplicated on partitions {0,1})
        ind_h = []
        for h in range(2):
            ih = const.tile([P, 2], F32, tag=f"ind{h}", name=f"ind{h}")
            nc.vector.memset(ih, 0.0)
            nc.vector.memset(ih[ts(h, Dh), :], 0.5)
            ind_h.append(ih)

        B_ORDER = ["bq_mu", "bq_var", "bk_mu", "bk_var", "bv_mu", "bv_var",
                   "bo_mu", "bo_var"]
        bcat0 = const.tile([CB, 8], F32, tag="bcat0", name="bcat0")
        nc.sync.dma_start(bcat0, b_d.ap())
        # bcat = bcat0 + 0 (gathered zeros): every downstream op that leads
        # to a collective input passes through an ACT/DVE op reading these
        # biases, so this dependency aligns all cores' streams.
        bcat = const.tile([CB, 8], F32, tag="bcat", name="bcat")
        nc.vector.tensor_scalar_add(bcat, bcat0, z128[:, 0:1])
        nbcat = const.tile([CB, 8], F32, tag="nbcat", name="nbcat")
        nc.vector.tensor_scalar_mul(nbcat, bcat, -1.0)
        bias = {n: bcat[:, i:i + 1] for i, n in enumerate(B_ORDER)}
        nbias = {n: nbcat[:, i:i + 1] for i, n in enumerate(B_ORDER)}

        # warm the sigmoid table while DMAs run
        warm_sg = stage.tile([1, 1], F32, tag="warm_sg", name="warm_sg",
                             bufs=1)
        nc.scalar.activation(warm_sg, ident_b[0:1, 0:1], AF.Sigmoid)

        # ---------------- projections (feature-major [CB, L], bf16) ----
        def project(wi, si):
            ps = ps_proj.tile([P, L], F32, tag="proj", name="proj")
            for kt in range(KT):
                nc.tensor.matmul(ps, w_sb[:, wi, kt, :], x_sb[:, si, kt, :],
                                 start=(kt == 0), stop=(kt == KT - 1))
            return ps

        # var side first: 3 sigmoids (one table residency), then one wide
        # Ln over all three at once (forces sig/sig/sig/ln queue order).
        sg_all = feat.tile([P, 3 * L], F32, tag="sg_all", name="sg_all")
        ps_qv = project(0, 0)
        nc.scalar.activation(sg_all[:, 0 * L:1 * L], ps_qv, AF.Sigmoid,
                             scale=-1.0, bias=nbias["bq_var"])
        ps_kv = project(1, 0)
        nc.scalar.activation(sg_all[:, 1 * L:2 * L], ps_kv, AF.Sigmoid,
                             scale=-1.0, bias=nbias["bk_var"])
        ps_vv = project(2, 0)
        nc.scalar.activation(sg_all[:, 2 * L:3 * L], ps_vv, AF.Sigmoid,
                             scale=-1.0, bias=nbias["bv_var"])

        # nsp_* = ln(sigmoid(-(x+b))) = -softplus(x+b)
        nsp_all = feat.tile([P, 3 * L], F32, tag="nsp_all", name="nsp_all")
        nc.scalar.activation(nsp_all, sg_all, AF.Ln)
        nsp_q = nsp_all[:, 0 * L:1 * L]
        nsp_k = nsp_all[:, 1 * L:2 * L]
        nsp_v = nsp_all[:, 2 * L:3 * L]

        t_kv = feat.tile([P, L], F32, tag="t_kv", name="t_kv")
        nc.vector.tensor_scalar_mul(t_kv, nsp_k, -1.0)    # Kv
        t_lg = feat.tile([P, L], F32, tag="t_lg", name="t_lg")
        nc.scalar.activation(t_lg, t_kv, AF.Ln)           # ln Kv

        # mu-side projections (DVE-only extraction)
        ps_qm = project(3, 1)
        t_qm = feat.tile([P, L], F32, tag="t_qm", name="t_qm")
        nc.vector.tensor_scalar_add(t_qm, ps_qm, bias["bq_mu"])
        negqm_bf = feat.tile([P, L], BF16, tag="negqm", name="negqm")
        nc.vector.tensor_scalar_mul(negqm_bf, t_qm, -1.0)
        t_qm2 = feat.tile([P, L], F32, tag="t_qm2", name="t_qm2")
        nc.vector.tensor_mul(t_qm2, t_qm, t_qm)
        ps_km = project(4, 1)
        t_km = feat.tile([P, L], F32, tag="t_km", name="t_km")
        nc.vector.tensor_scalar_add(t_km, ps_km, bias["bk_mu"])
        ps_vm = project(5, 1)
        vmT = feat.tile([P, L], BF16, tag="vmT", name="vmT")
        nc.vector.tensor_scalar_add(vmT, ps_vm, bias["bv_mu"])

        # derived features
        fq1_bf = feat.tile([P, L], BF16, tag="fq1", name="fq1")
        nc.vector.tensor_sub(fq1_bf, t_qm2, nsp_q)        # Qm^2 + Qv
        vvT = feat.tile([P, L], BF16, tag="vvT", name="vvT")
        nc.vector.tensor_scalar_mul(vvT, nsp_v, -1.0)     # Vv
        # 1/Kv = exp(-ln Kv) on ACT (exp table now resident for attention)
        t_iv = feat.tile([P, L], F32, tag="t_iv", name="t_iv")
        nc.scalar.activation(t_iv, t_lg, AF.Exp, scale=-1.0)
        fk1_bf = feat.tile([P, L], BF16, tag="fk1", name="fk1")
        nc.vector.tensor_scalar_mul(fk1_bf, t_iv, 0.5)
        t_kiv = feat.tile([P, L], F32, tag="t_kiv", name="t_kiv")
        nc.vector.tensor_mul(t_kiv, t_km, t_iv)
        kmiv_bf = feat.tile([P, L], BF16, tag="kmiv", name="kmiv")
        nc.vector.tensor_copy(kmiv_bf, t_kiv)
        t_u = feat.tile([P, L], F32, tag="t_u", name="t_u")
        nc.vector.tensor_mul(t_u, t_kiv, t_km)
        t_s = feat.tile([P, L], F32, tag="t_s", name="t_s")
        nc.vector.tensor_add(t_s, t_u, t_lg)

        # V to L-major bf16 via PE transpose
        v_l = {}
        for nm, src in (("vm", vmT), ("vv", vvT)):
            for lk in range(LT):
                pt = ps_tr.tile([P, P], BF16, tag="tr", name="trb")
                nc.tensor.transpose(pt, src[:, ts(lk, P)], ident_b)
                dst = feat.tile([P, P], BF16, tag=f"vl_{nm}_{lk}",
                                name=f"vl_{nm}_{lk}")
                nc.vector.tensor_copy(dst, pt)
                v_l[(nm, lk)] = dst

        # r per head: 0.5 * sum_d (Km^2/Kv + log Kv) (fp32 path). One
        # duplicate-row matmul per head puts r_h on partitions {0,1};
        # the bf16 hi/lo tiles keep both rows identical (no partition
        # shifts) and are added to the logits via K=2 matmuls against a
        # 0.5-valued lhsT (0.5*(v+v) = v).
        r_hi, r_lo = [], []
        for h in range(2):
            prh = ps_small.tile([2, L], F32, tag="r_ps",
                                name=f"r_ps{h}")
            nc.tensor.matmul(prh, ind_h[h], t_s, start=True, stop=True)
            hi2 = feat.tile([2, L], BF16, tag=f"r_hi{h}", name=f"r_hi{h}")
            nc.vector.tensor_copy(hi2, prh)
            lo2f = feat.tile([2, L], F32, tag=f"r_lof{h}", name=f"r_lof{h}")
            nc.vector.tensor_sub(lo2f, prh, hi2)
            lo2 = feat.tile([2, L], BF16, tag=f"r_lo{h}", name=f"r_lo{h}")
            nc.vector.tensor_copy(lo2, lo2f)
            r_hi.append(hi2)
            r_lo.append(lo2)

        # ---------------- attention ------------------------------------
        # kl >= 0 and max_k kl/8 is O(10) => exp without max-subtraction
        attnT = {}   # (h, lk) -> [128 (k within lk), 256 (q)] bf16
        a2T = {}     # squared attention (var path), built inline
        for h in range(2):
            hs = ts(h, Dh)
            for t in range(LT):
                ps_S = ps_s.tile([P, L], F32, tag="scores", name="scores")
                nc.tensor.matmul(ps_S, fq1_bf[hs, ts(t, P)], fk1_bf[hs, :],
                                 start=True, stop=False)
                nc.tensor.matmul(ps_S, negqm_bf[hs, ts(t, P)], kmiv_bf[hs, :],
                                 start=False, stop=False)
                nc.tensor.matmul(ps_S, half2, r_hi[h],
                                 start=False, stop=False)
                nc.tensor.matmul(ps_S, half2, r_lo[h],
                                 start=False, stop=True)
                pexp = attnp.tile([P, L], BF16, tag="pexp", name="pexp")
                den = attnp.tile([P, 1], F32, tag="den", name="den")
                nc.scalar.activation(pexp, ps_S, AF.Exp, bias=0.0,
                                     scale=-0.125, accum_out=den)
                invd = attnp.tile([P, 1], F32, tag="invd", name="invd")
                nc.vector.reciprocal(invd, den)
                a_bf = attnp.tile([P, L], BF16, tag="a_bf", name="a_bf")
                nc.vector.tensor_scalar_mul(a_bf, pexp, invd)
                for lk in range(LT):
                    if (h, lk) not in attnT:
                        attnT[(h, lk)] = feat.tile(
                            [P, L], BF16, tag=f"attnT_{h}_{lk}",
                            name=f"attnT_{h}_{lk}")
                        a2T[(h, lk)] = feat.tile(
                            [P, L], BF16, tag=f"a2T_{h}_{lk}",
                            name=f"a2T_{h}_{lk}")
                    pt = ps_tr.tile([P, P], BF16, tag="tr", name="trb")
                    nc.tensor.transpose(pt, a_bf[:, ts(lk, P)], ident_b)
                    nc.vector.tensor_copy(attnT[(h, lk)][:, ts(t, P)], pt)
                    nc.vector.tensor_mul(a2T[(h, lk)][:, ts(t, P)],
                                         attnT[(h, lk)][:, ts(t, P)],
                                         attnT[(h, lk)][:, ts(t, P)])

        # ---------------- PV + two pipelined AllGathers ----------------
        # var first: the mu PV/DMA and the var out-projection + epilogue
        # overlap the collectives' transfer time.
        cc_in_var = dram.tile([CB, L], BF16, tag="cc_in_var",
                              name="cc_in_var")
        cc_in_mu = dram.tile([CB, L], BF16, tag="cc_in_mu", name="cc_in_mu")
        pv_var = ps_small.tile([P, L], F32, tag="pv", name="pv_var", bufs=1)
        for h in range(2):
            for lk in range(LT):
                nc.tensor.matmul(pv_var[ts(h, Dh), :],
                                 v_l[("vv", lk)][:, ts(h, Dh)],
                                 a2T[(h, lk)],
                                 start=(lk == 0), stop=(lk == LT - 1),
                                 tile_position=(0, h * Dh))
        o_var = attnp.tile([P, L], BF16, tag="o_var", name="o_var")
        nc.vector.tensor_copy(o_var, pv_var)
        nc.sync.dma_start(cc_in_var[:, :], o_var)
        cc_out_var = dram.tile([NCORES * CB, L], BF16, tag="cc_out_var",
                               name="cc_out_var", addr_space="Shared")
        nc.gpsimd.collective_compute(
            "AllGather", ALU.bypass,
            replica_groups=[list(range(NCORES))],
            ins=[cc_in_var[:].opt()],
            outs=[cc_out_var[:].opt()],
        )

        pv_mu = ps_small.tile([P, L], F32, tag="pv", name="pv_mu", bufs=1)
        for h in range(2):
            for lk in range(LT):
                nc.tensor.matmul(pv_mu[ts(h, Dh), :],
                                 v_l[("vm", lk)][:, ts(h, Dh)],
                                 attnT[(h, lk)],
                                 start=(lk == 0), stop=(lk == LT - 1),
                                 tile_position=(0, h * Dh))
        o_mu = attnp.tile([P, L], BF16, tag="o_mu", name="o_mu")
        nc.vector.tensor_copy(o_mu, pv_mu)
        nc.sync.dma_start(cc_in_mu[:, :], o_mu)
        cc_out_mu = dram.tile([NCORES * CB, L], BF16, tag="cc_out_mu",
                              name="cc_out_mu", addr_space="Shared")
        nc.gpsimd.collective_compute(
            "AllGather", ALU.bypass,
            replica_groups=[list(range(NCORES))],
            ins=[cc_in_mu[:].opt()],
            outs=[cc_out_mu[:].opt()],
        )

        # ---------------- output projections ---------------------------
        gall = {}
        for half, cco in ((0, cc_out_var), (1, cc_out_mu)):
            g = stage.tile([P, KT, L], BF16, tag=f"gall_{half}",
                           name=f"gall_{half}", bufs=1)
            nc.sync.dma_start(g, cco.rearrange("(c p) m -> p c m", p=P))
            gall[half] = g

        def out_proj(wi, half):
            ps = ps_proj.tile([P, L], F32, tag="proj", name="proj")
            for kt in range(KT):
                nc.tensor.matmul(ps, w_sb[:, wi, kt, :],
                                 gall[half][:, kt, :],
                                 start=(kt == 0), stop=(kt == KT - 1))
            return ps

        # var first; softplus(x) = ln(1+exp(x)) reuses the resident exp
        # table, so the tail pays only ONE table load (the final Ln).
        ps_ovar = out_proj(6, 0)
        u_var = stage.tile([P, L], F32, tag="u_var", name="u_var", bufs=1)
        nc.scalar.activation(u_var, ps_ovar, AF.Exp, scale=1.0,
                             bias=bias["bo_var"])
        w1_var = stage.tile([P, L], F32, tag="w1_var", name="w1_var", bufs=1)
        nc.vector.tensor_scalar_add(w1_var, u_var, 1.0)
        ps_omu = out_proj(7, 1)
        res_var = stage.tile([P, L], F32, tag="res_var", name="res_var")
        nc.scalar.activation(res_var, w1_var, AF.Ln)
        nc.sync.dma_start(out_var_d.ap(), res_var)
        res_mu = stage.tile([P, L], F32, tag="res_mu", name="res_mu")
        nc.vector.tensor_scalar_add(res_mu, ps_omu, bias["bo_mu"])
        nc.sync.dma_start(out_mu_d.ap(), res_mu)


def shard_inputs(inputs):
    """Full inputs -> per-core in_maps (host-side numpy prep only)."""
    f32 = np.float32
    bf16 = mybir.dt.np(BF16)

    def to_pe_tiles(a):      # [1024, n] -> [128, 8, n]
        n = a.shape[1]
        return np.ascontiguousarray(
            a.reshape(KT, P, n).transpose(1, 0, 2))

    xcat = np.empty((P, 2, KT, L), dtype=bf16)
    for si, nm in enumerate(("var", "mu")):
        xt = np.asarray(inputs[nm]).reshape(L, D).astype(f32).T  # [D, L]
        xcat[:, si] = to_pe_tiles(xt.astype(bf16))

    W_ORDER = ["wq_var", "wk_var", "wv_var", "wq_mu", "wk_mu", "wv_mu",
               "wo_var", "wo_mu"]
    B_NAMES = ["bq_mu", "bq_var", "bk_mu", "bk_var", "bv_mu", "bv_var",
               "bo_mu", "bo_var"]
    in_maps = []
    for c in range(NCORES):
        cols = slice(c * CB, (c + 1) * CB)
        wcat = np.empty((P, NW, KT, CB), dtype=bf16)
        for wi, nm in enumerate(W_ORDER):
            w = np.asarray(inputs[nm])[:, cols].astype(f32).astype(bf16)
            wcat[:, wi] = to_pe_tiles(w)
        biases = np.ascontiguousarray(np.stack(
            [np.asarray(inputs[n])[cols].astype(f32) for n in B_NAMES],
            axis=1))
        in_maps.append({"xcat": xcat, "wcat": wcat, "biases": biases})
    return in_maps


def kernel(**inputs):
    global LAST_RESULT
    if "prog" not in _prog_cache:
        _prog_cache["prog"] = build_program()
    nc = _prog_cache["prog"]
    in_maps = shard_inputs(inputs)
    res = run_bass_kernel_spmd(nc, in_maps, core_ids=list(range(NCORES)),
                               trace=TRACE, **TRACE_KWARGS)
    LAST_RESULT = res
    mu_blocks = [res.results[c]["out_mu"] for c in range(NCORES)]
    var_blocks = [res.results[c]["out_var"] for c in range(NCORES)]
    mu_out = np.concatenate(mu_blocks, axis=0).T.reshape(B, L, D)
    var_out = np.concatenate(var_blocks, axis=0).T.reshape(B, L, D)
    return (np.ascontiguousarray(mu_out.astype(np.float32)),
            np.ascontiguousarray(var_out.astype(np.float32)))


# revision 19
# speedup vs baseline: 1.2684x; 1.2684x over previous
"""Distributional (Gaussian-KL) attention on 8 TRN2 NeuronCores.

Math: for each head, the KL-based score decomposes as
    kl[q,k] = sum_d (Qm-Km)^2/(2Kv) + 0.5*(Qv/Kv - log(Qv/Kv) - 1)
            = Fq[q] . Fk[k] + r[k] + c[q]
with  Fq = [Qm^2+Qv ; -Qm],  Fk = [1/(2Kv) ; Km/Kv],
      r  = 0.5*sum_d (Km^2/Kv + log Kv),
and c[q] only shifts softmax logits per-row (drops out of softmax over k).
scores = -kl/sqrt(Dh); kl >= 0 and measured row-max kl/8 <= ~17, so
exp(scores) needs no max-shift.

Sharding: head-parallel. Core c owns heads {2c, 2c+1} == feature columns
[128c, 128c+128) of every Q/K/V projection. The output projection needs
all heads, so the per-core attention outputs (feature-major, bf16, mu and
var packed side by side in one [128, 512] buffer) are AllGathered in a
SINGLE collective, after which core c computes output columns
[128c, 128c+128). Host concatenates + transposes.

Host-side prep (inside kernel(), numpy only): weights and inputs are
pre-cast to bf16 and pre-transposed/tiled into PE-friendly layouts, so
the device does no f32->bf16 casting and no input transposes at all.

Precision: all matmuls bf16 except the r_k bias path (true fp32 - r is
O(100+) and bf16 rounding there shifts logits by ~0.15; r is added to
the logits as a bf16 hi+lo pair via two rank-1 matmuls).

ACT table discipline (each table swap costs ~1.28us): the op order is
sigmoid x3 -> one wide Ln (+ t_lg Ln) -> exp (1/Kv, attention, final
softplus numerator) -> one final Ln.  softplus(x)=ln(sigmoid(-x)) up
front (sigmoid loaded first), but the OUTPUT softplus is computed as
ln(1+exp(x)) so it reuses the exp table that attention left resident.
1/Kv = exp(-ln Kv) on ACT replaces a slow DVE reciprocal.
"""

import numpy as np

import concourse.bass as bass
import concourse.mybir as mybir
import concourse.tile as tile
from concourse import bacc
from concourse.masks import make_identity
from concourse.bass_utils import run_bass_kernel_spmd

F32 = mybir.dt.float32
BF16 = mybir.dt.bfloat16
AF = mybir.ActivationFunctionType
ALU = mybir.AluOpType
AX = mybir.AxisListType

H, B, L, D = 16, 1, 256, 1024
Dh = D // H          # 64
NCORES = 8
CB = D // NCORES     # 128 feature columns per core (2 heads)
P = 128
LT = L // P          # 2 row tiles of the sequence
KT = D // P          # 8 contraction tiles
NW = 8               # weights in wcat: qv kv vv qm km vm | ov om

TRACE = False
TRACE_KWARGS = {}
LAST_RESULT = None

_prog_cache = {}


def ts(i, size):
    return slice(i * size, (i + 1) * size)


def build_program():
    nc = bacc.Bacc("TRN2", target_bir_lowering=False, debug=False,
                   num_devices=NCORES)

    # xcat[p, s, kt, l] = x_s[l, kt*128+p]; s: 0=var, 1=mu
    xcat_d = nc.dram_tensor("xcat", [P, 2, KT, L], BF16, kind="ExternalInput")
    # wcat[p, w, kt, m] = w[kt*128+p, m]
    wcat_d = nc.dram_tensor("wcat", [P, NW, KT, CB], BF16,
                            kind="ExternalInput")
    b_d = nc.dram_tensor("biases", [CB, 8], F32, kind="ExternalInput")
    out_mu_d = nc.dram_tensor("out_mu", [CB, L], F32, kind="ExternalOutput")
    out_var_d = nc.dram_tensor("out_var", [CB, L], F32, kind="ExternalOutput")

    with tile.TileContext(nc) as tc:
        _build(nc, tc, xcat_d, wcat_d, b_d, out_mu_d, out_var_d)
    nc.compile()
    return nc


def _build(nc, tc, xcat_d, wcat_d, b_d, out_mu_d, out_var_d):
    from contextlib import ExitStack
    ctx = ExitStack()
    with ctx:
        const = ctx.enter_context(tc.tile_pool(name="const", bufs=1))
        persist = ctx.enter_context(tc.tile_pool(name="persist", bufs=1))
        stage = ctx.enter_context(tc.tile_pool(name="stage", bufs=3))
        feat = ctx.enter_context(tc.tile_pool(name="feat", bufs=1))
        attnp = ctx.enter_context(tc.tile_pool(name="attnp", bufs=2))
        ps_proj = ctx.enter_context(
            tc.tile_pool(name="ps_proj", bufs=2, space="PSUM"))
        ps_tr = ctx.enter_context(
            tc.tile_pool(name="ps_tr", bufs=2, space="PSUM"))
        ps_s = ctx.enter_context(
            tc.tile_pool(name="ps_s", bufs=2, space="PSUM"))
        ps_small = ctx.enter_context(
            tc.tile_pool(name="ps_small", bufs=1, space="PSUM"))
        dram = ctx.enter_context(tc.tile_pool(name="dram", bufs=1,
                                              space="DRAM"))

        # ---------------- inputs: clean bf16 DMAs, finest useful order --
        x_sb = persist.tile([P, 2, KT, L], BF16, tag="x_sb", name="x_sb")
        nc.sync.dma_start(x_sb[:, 0], xcat_d.ap()[:, 0])     # var first
        w_sb = persist.tile([P, NW, KT, CB], BF16, tag="w_sb", name="w_sb")
        for wi in range(3):                                  # qkv_var
            nc.sync.dma_start(w_sb[:, wi:wi + 1], wcat_d.ap()[:, wi:wi + 1])
        nc.sync.dma_start(x_sb[:, 1], xcat_d.ap()[:, 1])     # mu
        for wi in range(3, 6):                               # qkv_mu
            nc.sync.dma_start(w_sb[:, wi:wi + 1], wcat_d.ap()[:, wi:wi + 1])
        nc.sync.dma_start(w_sb[:, 6:8], wcat_d.ap()[:, 6:8])  # wo_var, wo_mu

        # ---------------- constants -----------------------------------
        ident_b = const.tile([P, P], BF16, tag="ident_b", name="ident_b")
        make_identity(nc, ident_b)
        for _w in range(8):
            wmp = ps_tr.tile([P, P], F32, tag="tr", name="warm")
            nc.tensor.matmul(wmp, ident_b, ident_b, start=True, stop=True)
        half2 = const.tile([2, P], BF16, tag="half2", name="half2")
        nc.vector.memset(half2, 0.5)
        # ind_h[p, :] = 0.5 if p in head h else 0 (both cols; the fp32 r
        # matmul then yields r_h duplicated on partitions {0,1})
        ind_h = []
        for h in range(2):
            ih = const.tile([P, 2], F32, tag=f"ind{h}", name=f"ind{h}")
            nc.vector.memset(ih, 0.0)
            nc.vector.memset(ih[ts(h, Dh), :], 0.5)
            ind_h.append(ih)

        B_ORDER = ["bq_mu", "bq_var", "bk_mu", "bk_var", "bv_mu", "bv_var",
                   "bo_mu", "bo_var"]
        bcat = const.tile([CB, 8], F32, tag="bcat", name="bcat")
        nc.sync.dma_start(bcat, b_d.ap())
        nbcat = const.tile([CB, 8], F32, tag="nbcat", name="nbcat")
        nc.vector.tensor_scalar_mul(nbcat, bcat, -1.0)
        bias = {n: bcat[:, i:i + 1] for i, n in enumerate(B_ORDER)}
        nbias = {n: nbcat[:, i:i + 1] for i, n in enumerate(B_ORDER)}

        # warm the sigmoid table while DMAs run
        warm_sg = stage.tile([1, 1], F32, tag="warm_sg", name="warm_sg",
                             bufs=1)
        nc.scalar.activation(warm_sg, ident_b[0:1, 0:1], AF.Sigmoid)

        # ---------------- projections (feature-major [CB, L], bf16) ----
        def project(wi, si):
            ps = ps_proj.tile([P, L], F32, tag="proj", name="proj")
            for kt in range(KT):
                nc.tensor.matmul(ps, w_sb[:, wi, kt, :], x_sb[:, si, kt, :],
                                 start=(kt == 0), stop=(kt == KT - 1))
            return ps

        # var side first: 3 sigmoids (one table residency), then one wide
        # Ln over all three at once (forces sig/sig/sig/ln queue order).
        sg_all = feat.tile([P, 3 * L], F32, tag="sg_all", name="sg_all")
        ps_qv = project(0, 0)
        nc.scalar.activation(sg_all[:, 0 * L:1 * L], ps_qv, AF.Sigmoid,
                             scale=-1.0, bias=nbias["bq_var"])
        ps_kv = project(1, 0)
        nc.scalar.activation(sg_all[:, 1 * L:2 * L], ps_kv, AF.Sigmoid,
                             scale=-1.0, bias=nbias["bk_var"])
        ps_vv = project(2, 0)
        nc.scalar.activation(sg_all[:, 2 * L:3 * L], ps_vv, AF.Sigmoid,
                             scale=-1.0, bias=nbias["bv_var"])

        # nsp_* = ln(sigmoid(-(x+b))) = -softplus(x+b)
        nsp_all = feat.tile([P, 3 * L], F32, tag="nsp_all", name="nsp_all")
        nc.scalar.activation(nsp_all, sg_all, AF.Ln)
        nsp_q = nsp_all[:, 0 * L:1 * L]
        nsp_k = nsp_all[:, 1 * L:2 * L]
        nsp_v = nsp_all[:, 2 * L:3 * L]

        t_kv = feat.tile([P, L], F32, tag="t_kv", name="t_kv")
        nc.vector.tensor_scalar_mul(t_kv, nsp_k, -1.0)    # Kv
        t_lg = feat.tile([P, L], F32, tag="t_lg", name="t_lg")
        nc.scalar.activation(t_lg, t_kv, AF.Ln)           # ln Kv

        # mu-side projections (DVE-only extraction)
        ps_qm = project(3, 1)
        t_qm = feat.tile([P, L], F32, tag="t_qm", name="t_qm")
        nc.vector.tensor_scalar_add(t_qm, ps_qm, bias["bq_mu"])
        negqm_bf = feat.tile([P, L], BF16, tag="negqm", name="negqm")
        nc.vector.tensor_scalar_mul(negqm_bf, t_qm, -1.0)
        t_qm2 = feat.tile([P, L], F32, tag="t_qm2", name="t_qm2")
        nc.vector.tensor_mul(t_qm2, t_qm, t_qm)
        ps_km = project(4, 1)
        t_km = feat.tile([P, L], F32, tag="t_km", name="t_km")
        nc.vector.tensor_scalar_add(t_km, ps_km, bias["bk_mu"])
        ps_vm = project(5, 1)
        vmT = feat.tile([P, L], BF16, tag="vmT", name="vmT")
        nc.vector.tensor_scalar_add(vmT, ps_vm, bias["bv_mu"])

        # derived features
        fq1_bf = feat.tile([P, L], BF16, tag="fq1", name="fq1")
        nc.vector.tensor_sub(fq1_bf, t_qm2, nsp_q)        # Qm^2 + Qv
        vvT = feat.tile([P, L], BF16, tag="vvT", name="vvT")
        nc.vector.tensor_scalar_mul(vvT, nsp_v, -1.0)     # Vv
        # 1/Kv = exp(-ln Kv) on ACT (exp table now resident for attention)
        t_iv = feat.tile([P, L], F32, tag="t_iv", name="t_iv")
        nc.scalar.activation(t_iv, t_lg, AF.Exp, scale=-1.0)
        fk1_bf = feat.tile([P, L], BF16, tag="fk1", name="fk1")
        nc.vector.tensor_scalar_mul(fk1_bf, t_iv, 0.5)
        t_kiv = feat.tile([P, L], F32, tag="t_kiv", name="t_kiv")
        nc.vector.tensor_mul(t_kiv, t_km, t_iv)
        kmiv_bf = feat.tile([P, L], BF16, tag="kmiv", name="kmiv")
        nc.vector.tensor_copy(kmiv_bf, t_kiv)
        t_u = feat.tile([P, L], F32, tag="t_u", name="t_u")
        nc.vector.tensor_mul(t_u, t_kiv, t_km)
        t_s = feat.tile([P, L], F32, tag="t_s", name="t_s")
        nc.vector.tensor_add(t_s, t_u, t_lg)

        # V to L-major bf16 via PE transpose
        v_l = {}
        for nm, src in (("vm", vmT), ("vv", vvT)):
            for lk in range(LT):
                pt = ps_tr.tile([P, P], BF16, tag="tr", name="trb")
                nc.tensor.transpose(pt, src[:, ts(lk, P)], ident_b)
                dst = feat.tile([P, P], BF16, tag=f"vl_{nm}_{lk}",
                                name=f"vl_{nm}_{lk}")
                nc.vector.tensor_copy(dst, pt)
                v_l[(nm, lk)] = dst

        # r per head: 0.5 * sum_d (Km^2/Kv + log Kv) (fp32 path). One
        # duplicate-row matmul per head puts r_h on partitions {0,1};
        # the bf16 hi/lo tiles keep both rows identical (no partition
        # shifts) and are added to the logits via K=2 matmuls against a
        # 0.5-valued lhsT (0.5*(v+v) = v).
        r_hi, r_lo = [], []
        for h in range(2):
            prh = ps_small.tile([2, L], F32, tag="r_ps",
                                name=f"r_ps{h}")
            nc.tensor.matmul(prh, ind_h[h], t_s, start=True, stop=True)
            hi2 = feat.tile([2, L], BF16, tag=f"r_hi{h}", name=f"r_hi{h}")
            nc.vector.tensor_copy(hi2, prh)
            lo2f = feat.tile([2, L], F32, tag=f"r_lof{h}", name=f"r_lof{h}")
            nc.vector.tensor_sub(lo2f, prh, hi2)
            lo2 = feat.tile([2, L], BF16, tag=f"r_lo{h}", name=f"r_lo{h}")
            nc.vector.tensor_copy(lo2, lo2f)
            r_hi.append(hi2)
            r_lo.append(lo2)

        # ---------------- attention ------------------------------------
        # kl >= 0 and max_k kl/8 is O(10) => exp without max-subtraction
        attnT = {}   # (h, lk) -> [128 (k within lk), 256 (q)] bf16
        a2T = {}     # squared attention (var path), built inline
        for h in range(2):
            hs = ts(h, Dh)
            for t in range(LT):
                ps_S = ps_s.tile([P, L], F32, tag="scores", name="scores")
                nc.tensor.matmul(ps_S, fq1_bf[hs, ts(t, P)], fk1_bf[hs, :],
                                 start=True, stop=False)
                nc.tensor.matmul(ps_S, negqm_bf[hs, ts(t, P)], kmiv_bf[hs, :],
                                 start=False, stop=False)
                nc.tensor.matmul(ps_S, half2, r_hi[h],
                                 start=False, stop=False)
                nc.tensor.matmul(ps_S, half2, r_lo[h],
                                 start=False, stop=True)
                pexp = attnp.tile([P, L], BF16, tag="pexp", name="pexp")
                den = attnp.tile([P, 1], F32, tag="den", name="den")
                nc.scalar.activation(pexp, ps_S, AF.Exp, bias=0.0,
                                     scale=-0.125, accum_out=den)
                invd = attnp.tile([P, 1], F32, tag="invd", name="invd")
                nc.vector.reciprocal(invd, den)
                a_bf = attnp.tile([P, L], BF16, tag="a_bf", name="a_bf")
                nc.vector.tensor_scalar_mul(a_bf, pexp, invd)
                for lk in range(LT):
                    if (h, lk) not in attnT:
                        attnT[(h, lk)] = feat.tile(
                            [P, L], BF16, tag=f"attnT_{h}_{lk}",
                            name=f"attnT_{h}_{lk}")
                        a2T[(h, lk)] = feat.tile(
                            [P, L], BF16, tag=f"a2T_{h}_{lk}",
                            name=f"a2T_{h}_{lk}")
                    pt = ps_tr.tile([P, P], BF16, tag="tr", name="trb")
                    nc.tensor.transpose(pt, a_bf[:, ts(lk, P)], ident_b)
                    nc.vector.tensor_copy(attnT[(h, lk)][:, ts(t, P)], pt)
                    nc.vector.tensor_mul(a2T[(h, lk)][:, ts(t, P)],
                                         attnT[(h, lk)][:, ts(t, P)],
                                         attnT[(h, lk)][:, ts(t, P)])

        # ---------------- PV + two pipelined AllGathers ----------------
        # var first: the mu PV/DMA and the var out-projection + epilogue
        # overlap the collectives' transfer time.
        cc_in_var = dram.tile([CB, L], BF16, tag="cc_in_var",
                              name="cc_in_var")
        cc_in_mu = dram.tile([CB, L], BF16, tag="cc_in_mu", name="cc_in_mu")
        pv_var = ps_small.tile([P, L], F32, tag="pv", name="pv_var", bufs=1)
        for h in range(2):
            for lk in range(LT):
                nc.tensor.matmul(pv_var[ts(h, Dh), :],
                                 v_l[("vv", lk)][:, ts(h, Dh)],
                                 a2T[(h, lk)],
                                 start=(lk == 0), stop=(lk == LT - 1),
                                 tile_position=(0, h * Dh))
        o_var = attnp.tile([P, L], BF16, tag="o_var", name="o_var")
        nc.vector.tensor_copy(o_var, pv_var)
        nc.sync.dma_start(cc_in_var[:, :], o_var)
        cc_out_var = dram.tile([NCORES * CB, L], BF16, tag="cc_out_var",
                               name="cc_out_var", addr_space="Shared")
        nc.gpsimd.collective_compute(
            "AllGather", ALU.bypass,
            replica_groups=[list(range(NCORES))],
            ins=[cc_in_var[:].opt()],
            outs=[cc_out_var[:].opt()],
        )

        pv_mu = ps_small.tile([P, L], F32, tag="pv", name="pv_mu", bufs=1)
        for h in range(2):
            for lk in range(LT):
                nc.tensor.matmul(pv_mu[ts(h, Dh), :],
                                 v_l[("vm", lk)][:, ts(h, Dh)],
                                 attnT[(h, lk)],
                                 start=(lk == 0), stop=(lk == LT - 1),
                                 tile_position=(0, h * Dh))
        o_mu = attnp.tile([P, L], BF16, tag="o_mu", name="o_mu")
        nc.vector.tensor_copy(o_mu, pv_mu)
        nc.sync.dma_start(cc_in_mu[:, :], o_mu)
        cc_out_mu = dram.tile([NCORES * CB, L], BF16, tag="cc_out_mu",
                              name="cc_out_mu", addr_space="Shared")
        nc.gpsimd.collective_compute(
            "AllGather", ALU.bypass,
            replica_groups=[list(range(NCORES))],
            ins=[cc_in_mu[:].opt()],
            outs=[cc_out_mu[:].opt()],
        )

        # ---------------- output projections ---------------------------
        gall = {}
        for half, cco in ((0, cc_out_var), (1, cc_out_mu)):
            g = stage.tile([P, KT, L], BF16, tag=f"gall_{half}",
                           name=f"gall_{half}", bufs=1)
            nc.sync.dma_start(g, cco.rearrange("(c p) m -> p c m", p=P))
            gall[half] = g

        def out_proj(wi, half):
            ps = ps_proj.tile([P, L], F32, tag="proj", name="proj")
            for kt in range(KT):
                nc.tensor.matmul(ps, w_sb[:, wi, kt, :],
                                 gall[half][:, kt, :],
                                 start=(kt == 0), stop=(kt == KT - 1))
            return ps

        # var first; softplus(x) = ln(1+exp(x)) reuses the resident exp
        # table, so the tail pays only ONE table load (the final Ln).
        ps_ovar = out_proj(6, 0)
        u_var = stage.tile([P, L], F32, tag="u_var", name="u_var", bufs=1)
        nc.scalar.activation(u_var, ps_ovar, AF.Exp, scale=1.0,
                             bias=bias["bo_var"])
        w1_var = stage.tile([P, L], F32, tag="w1_var", name="w1_var", bufs=1)
        nc.vector.tensor_scalar_add(w1_var, u_var, 1.0)
        ps_omu = out_proj(7, 1)
        res_var = stage.tile([P, L], F32, tag="res_var", name="res_var")
        nc.scalar.activation(res_var, w1_var, AF.Ln)
        nc.sync.dma_start(out_var_d.ap(), res_var)
        res_mu = stage.tile([P, L], F32, tag="res_mu", name="res_mu")
        nc.vector.tensor_scalar_add(res_mu, ps_omu, bias["bo_mu"])
        nc.sync.dma_start(out_mu_d.ap(), res_mu)


def shard_inputs(inputs):
    """Full inputs -> per-core in_maps (host-side numpy prep only)."""
    f32 = np.float32
    bf16 = mybir.dt.np(BF16)

    def to_pe_tiles(a):      # [1024, n] -> [128, 8, n]
        n = a.shape[1]
        return np.ascontiguousarray(
            a.reshape(KT, P, n).transpose(1, 0, 2))

    xcat = np.empty((P, 2, KT, L), dtype=bf16)
    for si, nm in enumerate(("var", "mu")):
        xt = np.asarray(inputs[nm]).reshape(L, D).astype(f32).T  # [D, L]
        xcat[:, si] = to_pe_tiles(xt.astype(bf16))

    W_ORDER = ["wq_var", "wk_var", "wv_var", "wq_mu", "wk_mu", "wv_mu",
               "wo_var", "wo_mu"]
    B_NAMES = ["bq_mu", "bq_var", "bk_mu", "bk_var", "bv_mu", "bv_var",
               "bo_mu", "bo_var"]
    in_maps = []
    for c in range(NCORES):
        cols = slice(c * CB, (c + 1) * CB)
        wcat = np.empty((P, NW, KT, CB), dtype=bf16)
        for wi, nm in enumerate(W_ORDER):
            w = np.asarray(inputs[nm])[:, cols].astype(f32).astype(bf16)
            wcat[:, wi] = to_pe_tiles(w)
        biases = np.ascontiguousarray(np.stack(
            [np.asarray(inputs[n])[cols].astype(f32) for n in B_NAMES],
            axis=1))
        in_maps.append({"xcat": xcat, "wcat": wcat, "biases": biases})
    return in_maps


def kernel(**inputs):
    global LAST_RESULT
    if "prog" not in _prog_cache:
        _prog_cache["prog"] = build_program()
    nc = _prog_cache["prog"]
    in_maps = shard_inputs(inputs)
    res = run_bass_kernel_spmd(nc, in_maps, core_ids=list(range(NCORES)),
                               trace=TRACE, **TRACE_KWARGS)
    LAST_RESULT = res
    mu_blocks = [res.results[c]["out_mu"] for c in range(NCORES)]
    var_blocks = [res.results[c]["out_var"] for c in range(NCORES)]
    mu_out = np.concatenate(mu_blocks, axis=0).T.reshape(B, L, D)
    var_out = np.concatenate(var_blocks, axis=0).T.reshape(B, L, D)
    return (np.ascontiguousarray(mu_out.astype(np.float32)),
            np.ascontiguousarray(var_out.astype(np.float32)))
